# revision 39
# baseline (speedup 1.0000x reference)
"""TRN2 Bass kernel for nn_Aij (GAT-style dense attention coefficients).

Math (H=1 collapses the reference):
    s[b,i] = (encode[b,i,:] @ W) @ v_self      (scalar per node)
    n[b,j] = (encode[b,j,:] @ W) @ v_neigh     (scalar per node)
    out[b,i,j] = softmax_j( leaky_relu(s[b,i] + n[b,j], 0.2) )

Output is [8, 2048, 2048] -> memory-bound on the output store. Sharding:
data-parallel over batch; core b computes batch b.

Store-traffic optimization: the output is stored as fp16 (8 MiB/core instead
of 16 MiB), halving the DMA-bound store time. A global x512 scale (folded
into the exp biases host-side; divided back out on the host) keeps every
coefficient well inside fp16's normal range, so per-element relative error
stays at the ~5e-4 fp16 rounding floor, far inside the 2e-2 gate.

Compute: with exact host rowsums (bias_i = -ln S_i + ln 512), each element is
    out'[i,j] = exp(lrelu(s_i+n_j) + bias_i)
              = max( u_i*v_j, p_i*q_j ),     u = e^{s+bias}, v = e^{n},
                                             p = e^{0.2s+bias}, q = e^{0.2n}
(exp is monotone, lrelu(t) = max(t, 0.2t)). This turns the elementwise
softmax into two rank-1 products plus a max, which splits across engines
(each stays under the 1456 ns/tile fp16 store rate):

  - PE   : per row-tile, one PSUM tile pair holds p_i*q_j for the DVE
           columns (K=6 cross-term bf16-split matmul, fp32-accurate) and
           t = s_i+n_j for the ACT columns (baseline-style K=6 split).
           Tiny dummy matmuls at t=0 start the PE p-state ramp clock early.
  - DVE  : cols [0:CD): one fused stt  out = (vb * u_i) max PSUM_pq -> fp16
           (vb = fp16 broadcast of v; u_i per-partition scalar; branch1 in
           fp32 on the fly, branch2 from the PE). This is the kernel's
           critical line: 16 x ~1.72us of back-to-back stt.
  - ACT  : cols [CD:N): Prelu(psum_t) then Exp(+bias) -> fp16 (2 passes).
           (The GPSIMD/Pool engine has no legal ALU ops on real TRN2
           silicon — only DVE and ACT can do elementwise work; Pool only
           triggers one SWDGE load.)
  - DMA  : one fp16 store per row tile; loads are spread over the SP/ACT/
           gpsimd DGE queues, ordered to unblock the first tiles ASAP.

Tile 0 is split into four chunks spread over DVE/Pool/ACT with per-chunk
stores so the store stream starts as early as possible; the last tile
splits its store in halves to shorten the tail.
"""

import numpy as np
from ml_dtypes import bfloat16

B, N, F = 8, 2048, 64
P = 128  # partitions
NT = N // P  # 16 row tiles

# column split: DVE | ACT (the GPSIMD/Pool engine has no legal ALU ops on
# real TRN2 silicon, so only DVE and ACT can do elementwise work)
CD = 1536
CA = N - CD  # 512

LOG_SCALE = float(np.log(512.0))  # global output scale, divided out on host

_compiled = None


def _build():
    from contextlib import ExitStack

    import concourse.bacc as bacc
    import concourse.mybir as mybir
    import concourse.tile as tile

    F32 = mybir.dt.float32
    F16 = mybir.dt.float16
    BF16 = mybir.dt.bfloat16

    nc = bacc.Bacc("TRN2", target_bir_lowering=False)

    # packs: rows 0:6 = t-pack (rhs n-splits cols 0:N, lhsT s-splits N:2N)
    #        rows 6:12 = pq-pack (rhs q-splits cols 0:N, lhsT p-splits N:2N)
    packs = nc.dram_tensor("packs", [12, 2 * N], BF16, kind="ExternalInput")
    # scal: cols 0:NT = u, NT:2NT = exp biases (incl. ln 512), 2NT:3NT = p
    scal = nc.dram_tensor("scal", [P, 3 * NT], F32, kind="ExternalInput")
    # vbp: v broadcast to all partitions, fp16, cols [0:CD)
    vbp = nc.dram_tensor("vbp", [P, CD], F16, kind="ExternalInput")
    out = nc.dram_tensor("out", [N, N], F16, kind="ExternalOutput")

    AT = mybir.ActivationFunctionType
    ALU = mybir.AluOpType

    with tile.TileContext(nc) as tc, ExitStack() as ctx:
        singles = ctx.enter_context(tc.tile_pool(name="singles", bufs=1))
        psum = ctx.enter_context(tc.tile_pool(name="psum", bufs=2, space="PSUM"))
        lp = ctx.enter_context(tc.tile_pool(name="lp", bufs=3))
        outp = ctx.enter_context(tc.tile_pool(name="outp", bufs=8))

        # matmul operands need base partition 0/32/64: t-pack at rows 0:6,
        # pq-pack at rows 32:38. Loads spread over three DGE queues, ordered
        # so the tensors gating the first tiles land first.
        pk = singles.tile([38, 2 * N], BF16, tag="pk")
        sc = singles.tile([P, 3 * NT], F32, tag="sc")
        vb = singles.tile([P, CD], F16, tag="vb")
        nc.sync.dma_start(out=pk[32:38, :], in_=packs[6:12, :])
        nc.sync.dma_start(out=vb[:, 0:512], in_=vbp[:, 0:512])
        nc.sync.dma_start(out=vb[:, 512:CD], in_=vbp[:, 512:CD])
        nc.scalar.dma_start(out=pk[0:6, :], in_=packs[0:6, :])
        nc.gpsimd.dma_start(out=sc, in_=scal[:, :])

        # tiny dummy matmuls with no load dependencies start the PE p-state
        # ramp clock immediately
        wz = singles.tile([2, 640], BF16, tag="wz")
        nc.vector.memset(wz, 1.0)

        tpk = pk[0:6, :]
        qpk = pk[32:38, :]

        def mm_pq(pt, po, k, c0, c1):
            nc.tensor.matmul(
                pt[:, c0 - po : c1 - po],
                qpk[:, N + P * k : N + P * (k + 1)],
                qpk[:, c0:c1],
                start=True,
                stop=True,
            )

        def mm_t(pt, po, k, c0, c1):
            nc.tensor.matmul(
                pt[:, c0 - po : c1 - po],
                tpk[:, N + P * k : N + P * (k + 1)],
                tpk[:, c0:c1],
                start=True,
                stop=True,
            )

        def stt_psum(ot, pt, po, k, c0, c1):
            nc.vector.scalar_tensor_tensor(
                out=ot[:, c0:c1],
                in0=vb[:, c0:c1],
                scalar=sc[:, k : k + 1],
                in1=pt[:, c0 - po : c1 - po],
                op0=ALU.mult,
                op1=ALU.max,
            )

        def act_path(ot, k, c0, c1, regions):
            # regions: list of (psum_tile, psum_off, r0, r1) covering [c0,c1)
            lt = lp.tile([P, c1 - c0], F32, tag="lt")
            for pt, po, r0, r1 in regions:
                nc.scalar.activation(
                    out=lt[:, r0 - c0 : r1 - c0],
                    in_=pt[:, r0 - po : r1 - po],
                    func=AT.Prelu,
                    bias=0.0,
                    scale=1.0,
                    alpha=0.2,
                )
            nc.scalar.activation(
                out=ot[:, c0:c1],
                in_=lt,
                func=AT.Exp,
                bias=sc[:, NT + k : NT + k + 1],
                scale=1.0,
            )

        # per tile: PSUM is two independent tiles (3 banks for the DVE
        # columns' p*q, 1 bank for the ACT columns' t) so the DVE and ACT
        # paths recycle independently
        B_TILES = frozenset((7, 11))
        for k in range(NT):
            pt0 = psum.tile([P, CD], F32, tag="pt0")
            pt1 = psum.tile([P, CA], F32, tag="pt1")
            ot = outp.tile([P, N], F16, tag="ot")

            if k == 0:
                # startup tile: DVE gets only two chunks (its steady line is
                # the kernel's critical path); ACT absorbs [1024:2048) via a
                # dual-region prelu. Per-chunk stores start the stream early.
                for c in range(2):
                    nc.tensor.matmul(
                        pt0[:, 0:512], wz[0:2, 0:128], wz[0:2, 128:640],
                        start=True, stop=True,
                    )
                mm_pq(pt0, 0, k, 0, 512)
                stt_psum(ot, pt0, 0, k, 0, 512)
                nc.sync.dma_start(out=out[0:P, 0:512], in_=ot[:, 0:512])
                mm_pq(pt0, 0, k, 512, 1024)
                mm_t(pt0, 0, k, 1024, CD)
                mm_t(pt1, CD, k, CD, 2048)
                act_path(ot, k, 1024, 2048,
                         [(pt0, 0, 1024, CD), (pt1, CD, CD, 2048)])
                stt_psum(ot, pt0, 0, k, 512, 1024)
                nc.sync.dma_start(out=out[0:P, 512:1024], in_=ot[:, 512:1024])
                nc.sync.dma_start(out=out[0:P, 1024:2048], in_=ot[:, 1024:2048])
                continue

            if k in B_TILES:
                # "B" tile, same PSUM tile shapes but content rebalanced:
                # pq only in [0:1024); t fills pt0's last bank [1024:1536)
                # plus pt1. ACT (which has slack) absorbs 1024 columns,
                # shortening the DVE critical line by ~530 ns per B tile.
                mm_pq(pt0, 0, k, 0, 512)
                mm_pq(pt0, 0, k, 512, 1024)
                mm_t(pt0, 0, k, 1024, CD)
                mm_t(pt1, CD, k, CD, 2048)
                act_path(ot, k, 1024, 2048,
                         [(pt0, 0, 1024, CD), (pt1, CD, CD, 2048)])
                stt_psum(ot, pt0, 0, k, 0, 1024)
                nc.sync.dma_start(out=out[P * k : P * (k + 1), :], in_=ot)
                continue

            mm_pq(pt0, 0, k, 0, 512)
            mm_pq(pt0, 0, k, 512, 1024)
            mm_pq(pt0, 0, k, 1024, CD)
            mm_t(pt1, CD, k, CD, 2048)

            act_path(ot, k, CD, 2048, [(pt1, CD, CD, 2048)])

            if k == NT - 1:
                # tail: chunk the final stt so stores overlap the compute;
                # the ACT half's store rides the idle scalar queue
                r0 = P * k
                nc.scalar.dma_start(
                    out=out[r0 : r0 + P, CD:N], in_=ot[:, CD:N]
                )
                stt_psum(ot, pt0, 0, k, 0, 512)
                stt_psum(ot, pt0, 0, k, 512, 1024)
                stt_psum(ot, pt0, 0, k, 1024, CD)
                nc.sync.dma_start(out=out[r0 : r0 + P, 0:512], in_=ot[:, 0:512])
                nc.sync.dma_start(
                    out=out[r0 : r0 + P, 512:1024], in_=ot[:, 512:1024]
                )
                nc.scalar.dma_start(
                    out=out[r0 : r0 + P, 1024:CD], in_=ot[:, 1024:CD]
                )
            else:
                stt_psum(ot, pt0, 0, k, 0, CD)
                nc.sync.dma_start(out=out[P * k : P * (k + 1), :], in_=ot)

    nc.compile()
    return nc


def _get_compiled():
    global _compiled
    if _compiled is None:
        _compiled = _build()
    return _compiled


def _host_prep(encode, kernel, attn_kernel_self, attn_kernel_neighs):
    """Per-batch exp-domain vectors + packs for the device program."""
    enc = np.asarray(encode, np.float32)
    W = np.asarray(kernel, np.float32)[:, 0, :]
    v_s = np.asarray(attn_kernel_self, np.float32)[:, 0, 0]
    v_n = np.asarray(attn_kernel_neighs, np.float32)[:, 0, 0]

    # same association order as the reference: h = enc @ W, then h @ v
    h = enc.reshape(B * N, F) @ W
    s_all = (h @ v_s).reshape(B, N).astype(np.float32)
    n_all = (h @ v_n).reshape(B, N).astype(np.float32)

    def split3(x):
        hi = x.astype(bfloat16)
        lo = (x - hi.astype(np.float32)).astype(bfloat16)
        lo2 = (x - hi.astype(np.float32) - lo.astype(np.float32)).astype(bfloat16)
        return hi, lo, lo2

    in_maps = []
    for b in range(B):
        s, n = s_all[b], n_all[b]

        # exact rowsums: S_i = sum_j exp(lrelu(s_i + n_j)) via sorted split
        s64 = s.astype(np.float64)
        n64 = np.sort(n.astype(np.float64))
        suf = np.concatenate([np.cumsum(np.exp(n64)[::-1])[::-1], [0.0]])
        pre = np.concatenate([[0.0], np.cumsum(np.exp(0.2 * n64))])
        idx = np.searchsorted(n64, -s64, side="right")
        S = np.exp(s64) * suf[idx] + np.exp(0.2 * s64) * pre[idx]
        bias64 = -np.log(S) + LOG_SCALE

        u = np.exp(s64 + bias64).astype(np.float32)
        p = np.exp(0.2 * s64 + bias64).astype(np.float32)
        v = np.exp(n.astype(np.float64)).astype(np.float32)
        q = np.exp(0.2 * n.astype(np.float64)).astype(np.float32)

        s_sp, n_sp = split3(s), split3(n)
        p_sp, q_sp = split3(p), split3(q)

        packs = np.zeros((12, 2 * N), bfloat16)
        # t-pack: t = s_i + n_j
        for r in range(3):
            packs[r, 0:N] = bfloat16(1.0)
            packs[r, N:] = s_sp[r]
            packs[3 + r, 0:N] = n_sp[r]
            packs[3 + r, N:] = bfloat16(1.0)
        # pq-pack: p_i * q_j via 6 cross terms (drops O(2^-24) terms)
        lhs_rows = (p_sp[0], p_sp[0], p_sp[1], p_sp[0], p_sp[1], p_sp[2])
        rhs_rows = (q_sp[0], q_sp[1], q_sp[0], q_sp[2], q_sp[1], q_sp[0])
        for r in range(6):
            packs[6 + r, 0:N] = rhs_rows[r]
            packs[6 + r, N:] = lhs_rows[r]

        scal = np.empty((P, 3 * NT), np.float32)
        scal[:, 0:NT] = u.reshape(NT, P).T
        scal[:, NT : 2 * NT] = bias64.astype(np.float32).reshape(NT, P).T
        scal[:, 2 * NT :] = p.reshape(NT, P).T

        vbp = np.ascontiguousarray(
            np.broadcast_to(v[None, 0:CD], (P, CD))
        ).astype(np.float16)

        in_maps.append({"packs": packs, "scal": scal, "vbp": vbp})
    return in_maps


def kernel(encode, kernel, attn_kernel_self, attn_kernel_neighs):
    from concourse.bass_utils import run_bass_kernel_spmd

    in_maps = _host_prep(encode, kernel, attn_kernel_self, attn_kernel_neighs)
    nc = _get_compiled()
    res = run_bass_kernel_spmd(nc, in_maps, core_ids=list(range(B)))
    inv = np.float32(1.0 / 512.0)
    return np.stack(
        [res.results[b]["out"].astype(np.float32) * inv for b in range(B)]
    )
